# revision 11
# baseline (speedup 1.0000x reference)
"""Trainium2 Bass kernel for causal self-attention (B=4, T=2048, C=2048, H=16).

Sharding: 8 cores = 4 batches x 2 head-groups (8 heads each).
All-fp16 datapath (PSUM accumulation fp32). Per core:
  A) v = x @ Wv            -> fp16 spill [T, 1024]
  B+C merged, per head h:
    B-block: qkT features (q_h, k_h) = Wqk^T x^T + RoPE -> resident fp16
    C-block: flash-style SDPA, t-outer/j-inner, software-pipelined:
      scores mm -> exp (scalar) -> [diag tri-mul] -> p_sum += p (vector)
      -> PV mm accumulates numerator in PSUM (tensor, lagged 2 items)
      per t: ones-mm denominator from p_sum -> recip_approx (vector)
      -> y = psy * rden resident fp16
  D) partial_out = y^T @ wp -> [T, C] fp16 partial
Host sums core pairs per batch, adds b_proj and the folded bias row
bv @ wp (token-independent).
"""

import sys

import numpy as np

sys.path.insert(0, "/opt/trn_rl_repo")

import concourse.bass as bass  # noqa: E402,F401
import concourse.mybir as mybir  # noqa: E402
import concourse.tile as tile  # noqa: E402
from concourse import bacc  # noqa: E402

F32 = mybir.dt.float32
F16 = mybir.dt.float16
AF = mybir.ActivationFunctionType

B, T, C = 4, 2048, 2048
H, D = 16, 128
HPC = 8            # heads per core
P = 128
NT = 512           # matmul moving free dim
TT = T // NT       # 4 token tiles
CC = C // P        # 16 contraction chunks over C
NF = 2 * HPC       # 16 feature chunks, interleaved (q_h, k_h) per head
ROPE_BASE = 10000.0

_CACHE = {}


def _mm(nc, out, lhsT, rhs, **kw):
    nc.tensor.matmul(out, lhsT, rhs, **kw)


def build_program():
    nc = bacc.Bacc(name="csa_tp3")

    xt = nc.dram_tensor("xt", (C, T), F16, kind="ExternalInput")
    wqk = nc.dram_tensor("wqk", (C, NF * P), F16, kind="ExternalInput")
    bqk = nc.dram_tensor("bqk", (P, NF), F32, kind="ExternalInput")
    wv = nc.dram_tensor("wv", (C, HPC * D), F16, kind="ExternalInput")
    cs = nc.dram_tensor("cs", (P, T), F16, kind="ExternalInput")
    sw = nc.dram_tensor("sw", (P, T), F16, kind="ExternalInput")
    tri = nc.dram_tensor("tri", (P, P), F16, kind="ExternalInput")
    onesm = nc.dram_tensor("onesm", (P, P), F16, kind="ExternalInput")
    wp = nc.dram_tensor("wp", (HPC * D, C), F16, kind="ExternalInput")
    out = nc.dram_tensor("out", (T, C), F16, kind="ExternalOutput")

    v_spill = nc.dram_tensor("v_spill", (T, HPC * D), F16, kind="Internal")
    y_spill = nc.dram_tensor("y_spill", (HPC * D, T), F16, kind="Internal")

    with tile.TileContext(nc) as tc:
        with tc.tile_pool(name="persist", bufs=1) as persist:
            # q/k resident: 16 features x [128, T] fp16 (64 KB/partition)
            qk_res = [
                persist.tile([P, T], F16, tag=f"qk{f}", name=f"qk{f}")
                for f in range(NF)
            ]
            cs_t = persist.tile([P, T], F16, tag="cs", name="cs")
            sw_t = persist.tile([P, T], F16, tag="sw", name="sw")
            bqk_t = persist.tile([P, NF], F32, tag="bqk", name="bqk")
            tri_t = persist.tile([P, P], F16, tag="tri", name="tri")
            ones_t = persist.tile([P, P], F16, tag="ones", name="ones")

            with tc.tile_pool(name="xt_res", bufs=1) as xt_res:
                xtt = [None] * CC  # [128, T] fp16 per c-chunk

                # ---------------- phase A: V (two n-passes) --------------
                with (
                    tc.tile_pool(name="wv_pool", bufs=1) as wv_pool,
                    tc.tile_pool(name="va_pool", bufs=1) as va_pool,
                    tc.tile_pool(name="psum_a", bufs=1, space="PSUM") as psum_a,
                ):
                    # interleaved issue: (wv_n0[c], xt[c]) pairs, split in
                    # half-columns so the first chain is fed ASAP
                    wvh = [[None] * CC, [None] * CC]
                    HN = NT // 2
                    for c in range(CC):
                        w_ = wv_pool.tile([P, NT], F16, tag=f"wva{c}",
                                          name=f"wva{c}")
                        x_ = xt_res.tile([P, T], F16, tag=f"x{c}",
                                         name=f"x{c}")
                        for hh in range(2):
                            nc.sync.dma_start(
                                w_[:, hh * HN:(hh + 1) * HN],
                                wv[c * P:(c + 1) * P, hh * HN:(hh + 1) * HN])
                            nc.sync.dma_start(
                                x_[:, hh * HN:(hh + 1) * HN],
                                xt[c * P:(c + 1) * P, hh * HN:(hh + 1) * HN])
                        wvh[0][c] = w_
                        xtt[c] = x_
                    # rest of x (t=1..3) in need-by order, then wv n=1
                    for tt_ in range(1, TT):
                        for c in range(CC):
                            nc.sync.dma_start(
                                xtt[c][:, tt_ * NT:(tt_ + 1) * NT],
                                xt[c * P:(c + 1) * P,
                                   tt_ * NT:(tt_ + 1) * NT])
                    for c in range(CC):
                        w_ = wv_pool.tile([P, NT], F16, tag=f"wvb{c}",
                                          name=f"wvb{c}")
                        nc.sync.dma_start(w_[:], wv[c * P:(c + 1) * P,
                                                    NT:2 * NT])
                        wvh[1][c] = w_
                    # constants
                    nc.sync.dma_start(cs_t[:], cs[:])
                    nc.sync.dma_start(sw_t[:], sw[:])
                    nc.sync.dma_start(bqk_t[:], bqk[:])
                    nc.sync.dma_start(tri_t[:], tri[:])
                    nc.sync.dma_start(ones_t[:], onesm[:])

                    for n in range(2):
                        for mtok in range(T // P):
                            msl = slice(mtok * P, (mtok + 1) * P)
                            ps = psum_a.tile([P, NT], F32, tag="psa",
                                             bufs=8, name="psa")
                            for c in range(CC):
                                _mm(nc, ps[:], xtt[c][:, msl], wvh[n][c][:],
                                    start=(c == 0), stop=(c == CC - 1))
                            vt = va_pool.tile([P, NT], F16, tag="vt",
                                              bufs=3, name="vt")
                            nc.scalar.copy(vt[:], ps[:])
                            nc.gpsimd.dma_start(
                                v_spill[mtok * P:(mtok + 1) * P,
                                        n * NT:(n + 1) * NT],
                                vt[:],
                            )

                # ------------- merged phases B + C, per head -------------
                with (
                    tc.tile_pool(name="wq_pool", bufs=1) as wq_pool,
                    tc.tile_pool(name="rp_pool", bufs=1) as rp_pool,
                    tc.tile_pool(name="vh_pool", bufs=1) as vh_pool,
                    tc.tile_pool(name="sd_pool", bufs=1) as sd_pool,
                    tc.tile_pool(name="psum_bc", bufs=1,
                                 space="PSUM") as psum_bc,
                ):
                    hd = D // 2

                    def load_wq(fg):
                        tiles = []
                        for c in range(CC):
                            w_ = wq_pool.tile([P, 2 * P], F16,
                                              tag=f"wq{c}", bufs=2,
                                              name=f"wq{c}")
                            nc.sync.dma_start(
                                w_[:],
                                wqk[c * P:(c + 1) * P,
                                    fg * 2 * P:(fg + 1) * 2 * P],
                            )
                            tiles.append(w_)
                        return tiles

                    vh_t = [None] * HPC

                    def load_vh(h):
                        vh3 = vh_pool.tile([P, T // P, P], F16,
                                           tag="vh", bufs=3, name="vh3")
                        nc.sync.dma_start(
                            vh3[:],
                            v_spill[:, h * D:(h + 1) * D].rearrange(
                                "(j p) d -> p j d", p=P),
                        )
                        vh_t[h] = vh3

                    # C-block software pipeline (global across heads)
                    state = {}    # (h,t) -> (psy, p_sum)
                    pending = []  # [(h,t,j,nj,p,off)]
                    LOOK = 2

                    def c_front(h, t, j, nj):
                        qh = qk_res[2 * h]
                        kh = qk_res[2 * h + 1]
                        diag = (j >= 4 * t)
                        off = (j - 4 * t) * P if diag else 0
                        qsl = slice(t * NT + off, (t + 1) * NT)
                        pss = psum_bc.tile([P, NT], F32, tag="pss",
                                           bufs=3, name="pss")
                        _mm(nc, pss[:, off:],
                            kh[:, j * P:(j + 1) * P],
                            qh[:, qsl], start=True, stop=True)
                        p = sd_pool.tile([P, NT], F16, tag="p",
                                         bufs=5, name="p")
                        nc.scalar.activation(
                            p[:, off:], pss[:, off:], AF.Exp)
                        if diag:
                            nc.vector.tensor_mul(
                                p[:, off:off + P],
                                p[:, off:off + P],
                                tri_t[:],
                            )
                        if j == 0:
                            psy = psum_bc.tile([P, NT], F32, tag="psy",
                                               bufs=2, name="psy")
                            p_sum = sd_pool.tile([P, NT], F16,
                                                 tag="p_sum", bufs=2,
                                                 name="p_sum")
                            state[(h, t)] = (psy, p_sum)
                            nc.vector.tensor_copy(state[(h, t)][1][:], p[:])
                        else:
                            p_sum = state[(h, t)][1]
                            nc.vector.tensor_add(
                                p_sum[:, off:], p_sum[:, off:], p[:, off:])
                        pending.append((h, t, j, nj, p, off))

                    def c_back():
                        h, t, j, nj, p, off = pending.pop(0)
                        psy, p_sum = state[(h, t)]
                        _mm(nc, psy[:, off:],
                            vh_t[h][:, j, :], p[:, off:],
                            start=(j == 0), stop=(j == nj - 1))
                        if j == nj - 1:
                            psd = psum_bc.tile([P, NT], F32, tag="psd",
                                               bufs=1, name="psd")
                            _mm(nc, psd[:], ones_t[:], p_sum[:],
                                start=True, stop=True)
                            rden = sd_pool.tile([P, NT], F32,
                                                tag="rden", bufs=2,
                                                name="rden")
                            nc.vector.reciprocal_approx_fast(
                                rden[:], psd[:])
                            yst = sd_pool.tile([P, NT], F16,
                                                tag="yst", bufs=2,
                                                name="yst")
                            nc.vector.tensor_mul(yst[:], psy[:], rden[:])
                            nc.gpsimd.dma_start(
                                y_spill[h * P:(h + 1) * P,
                                        t * NT:(t + 1) * NT],
                                yst[:])
                            del state[(h, t)]

                    wq_next = load_wq(0)
                    load_vh(0)
                    load_vh(1)
                    for h in range(HPC):
                        # ---- B-block: features q_h (f=0), k_h (f=1) ----
                        wq_t = wq_next
                        if h + 1 < HPC:
                            wq_next = load_wq(h + 1)
                        if h + 2 < HPC:
                            load_vh(h + 2)
                        for f in range(2):
                            feat = h * 2 + f
                            pst = [
                                psum_bc.tile([P, NT], F32, tag="psb",
                                             bufs=2, name="psb")
                                for _ in range(TT)
                            ]
                            for c in range(CC):
                                lhsT = wq_t[c][:, f * P:(f + 1) * P]
                                for t in range(TT):
                                    _mm(nc, pst[t][:], lhsT,
                                        xtt[c][:, t * NT:(t + 1) * NT],
                                        start=(c == 0),
                                        stop=(c == CC - 1))
                            for t in range(TT):
                                sl = slice(t * NT, (t + 1) * NT)
                                ps = pst[t]
                                raw = rp_pool.tile([P, NT], F16,
                                                   tag="raw", bufs=2,
                                                   name="raw")
                                nc.scalar.activation(
                                    raw[:], ps[:], AF.Identity,
                                    bias=bqk_t[:, feat:feat + 1],
                                )
                                rsw = rp_pool.tile([P, NT], F16,
                                                   tag="rsw", bufs=2,
                                                   name="rsw")
                                nc.scalar.activation(
                                    rsw[0:hd, :], ps[hd:P, :],
                                    AF.Identity,
                                    bias=bqk_t[hd:P, feat:feat + 1],
                                )
                                nc.scalar.activation(
                                    rsw[hd:P, :], ps[0:hd, :],
                                    AF.Identity,
                                    bias=bqk_t[0:hd, feat:feat + 1],
                                )
                                t1 = rp_pool.tile([P, NT], F16,
                                                  tag="rt1", bufs=2,
                                                  name="rt1")
                                t2 = rp_pool.tile([P, NT], F16,
                                                  tag="rt2", bufs=2,
                                                  name="rt2")
                                nc.vector.tensor_mul(
                                    t1[:], raw[:], cs_t[:, sl])
                                nc.vector.tensor_mul(
                                    t2[:], rsw[:], sw_t[:, sl])
                                nc.vector.tensor_add(
                                    qk_res[feat][:, sl], t1[:], t2[:])

                        # ---- C-block: SDPA for head h ----
                        for t in range(TT):
                            nj = 4 * t + 4
                            for j in range(nj):
                                c_front(h, t, j, nj)
                                if len(pending) > LOOK:
                                    c_back()
                    while pending:
                        c_back()

            # ------------- phase D: projection -------------
            with (
                tc.tile_pool(name="wp_pool", bufs=1) as wp_pool,
                tc.tile_pool(name="ym_pool", bufs=1) as ym_pool,
                tc.tile_pool(name="ot_pool", bufs=1) as ot_pool,
                tc.tile_pool(name="psum_d", bufs=1, space="PSUM") as psum_d,
            ):
                wp_t = []
                for hh in range(HPC):
                    w_ = wp_pool.tile([P, C], F16, tag=f"wp{hh}",
                                      name=f"wp{hh}")
                    for n in range(4):
                        nc.sync.dma_start(
                            w_[:, n * NT:(n + 1) * NT],
                            wp[hh * P:(hh + 1) * P, n * NT:(n + 1) * NT])
                    wp_t.append(w_)

                ym_t = [None] * (T // P)

                def load_ym(m):
                    ym = ym_pool.tile([P, HPC, P], F16, tag="ym",
                                      bufs=3, name="ym")
                    nc.sync.dma_start(
                        ym[:],
                        y_spill[:, m * P:(m + 1) * P].rearrange(
                            "(h d) t -> d h t", d=P),
                    )
                    ym_t[m] = ym

                load_ym(0)
                load_ym(1)
                for m in range(T // P):
                    if m + 2 < T // P:
                        load_ym(m + 2)
                    msl = slice(m * P, (m + 1) * P)
                    pso = [
                        psum_d.tile([P, NT], F32, tag=f"pso{n}",
                                    bufs=2, name=f"pso{n}")
                        for n in range(4)
                    ]
                    for hh in range(HPC):
                        lhsT = ym_t[m][:, hh, :]
                        for n in range(4):
                            _mm(nc, pso[n][:], lhsT,
                                wp_t[hh][:, n * NT:(n + 1) * NT],
                                start=(hh == 0),
                                stop=(hh == HPC - 1))
                    ot = ot_pool.tile([P, C], F16, tag="ot",
                                      bufs=2, name="ot")
                    for n in range(4):
                        nc.scalar.copy(
                            ot[:, n * NT:(n + 1) * NT], pso[n][:])
                        nc.gpsimd.dma_start(
                            out[msl, n * NT:(n + 1) * NT],
                            ot[:, n * NT:(n + 1) * NT])

    nc.finalize()
    return nc


def prep_inputs(x, w_attn, b_attn, w_proj, b_proj):
    """Build the 8 per-core input maps from full inputs."""
    x = np.asarray(x, dtype=np.float32)
    w_attn = np.asarray(w_attn, dtype=np.float32)
    b_attn = np.asarray(b_attn, dtype=np.float32)
    w_proj = np.asarray(w_proj, dtype=np.float32)

    scale = np.float32(1.0 / np.sqrt(D))

    inv_freq = 1.0 / (ROPE_BASE ** (np.arange(0, D, 2, dtype=np.float32) / D))
    tpos = np.arange(T, dtype=np.float32)
    ang = np.outer(tpos, inv_freq)  # [T, 64]
    cos_t, sin_t = np.cos(ang).T, np.sin(ang).T  # [64, T]
    cs = np.ascontiguousarray(
        np.concatenate([cos_t, cos_t], axis=0)).astype(np.float16)
    sw = np.ascontiguousarray(
        np.concatenate([-sin_t, sin_t], axis=0)).astype(np.float16)

    qq = np.arange(P)
    kk = np.arange(P)[:, None]
    tri = np.ascontiguousarray(
        (qq[None, :] >= kk).astype(np.float16))  # [128,128] causal triangle

    onesm = np.ones((P, P), dtype=np.float16)

    in_maps = []
    for core in range(8):
        b = core // 2
        hg = core % 2
        heads = list(range(hg * HPC, (hg + 1) * HPC))
        # interleaved feature order: (q_h, k_h) per head
        wqk_cols = []
        bqk_vals = []
        for h in heads:
            qcol = np.arange(h * D, (h + 1) * D)
            kcol = qcol + C
            wqk_cols.append(w_attn[:, qcol] * scale)
            wqk_cols.append(w_attn[:, kcol])
            bqk_vals.append(b_attn[qcol] * scale)
            bqk_vals.append(b_attn[kcol])
        wqk_s = np.ascontiguousarray(
            np.concatenate(wqk_cols, axis=1)).astype(np.float16)
        bqk_s = np.ascontiguousarray(
            np.stack(bqk_vals, axis=1)).astype(np.float32)  # [128, 16]

        vcols = np.concatenate(
            [np.arange(h * D, (h + 1) * D) for h in heads]) + 2 * C
        wv_s = np.ascontiguousarray(w_attn[:, vcols]).astype(np.float16)
        pcols = np.concatenate(
            [np.arange(h * D, (h + 1) * D) for h in heads])
        wp_s = np.ascontiguousarray(w_proj[pcols, :]).astype(np.float16)
        xt_s = np.ascontiguousarray(x[b].T).astype(np.float16)

        in_maps.append({
            "xt": xt_s, "wqk": wqk_s, "bqk": bqk_s, "wv": wv_s,
            "cs": cs, "sw": sw, "tri": tri, "onesm": onesm, "wp": wp_s,
        })
    return in_maps


def _get_program():
    if "nc" not in _CACHE:
        _CACHE["nc"] = build_program()
    return _CACHE["nc"]


def _postprocess(outs, b_proj, bvp):
    # bvp[hg]: bv_core @ wp_core for head-group hg — the attention value
    # bias contributes a token-independent row to the projection output.
    base = np.asarray(b_proj, dtype=np.float32) + bvp[0] + bvp[1]
    return np.stack(
        [outs[2 * b].astype(np.float32) + outs[2 * b + 1].astype(np.float32)
         + base[None, :] for b in range(B)]
    ).astype(np.float32)


def _run(inputs, trace=False):
    from concourse.bass_utils import run_bass_kernel_spmd

    nc = _get_program()
    in_maps = prep_inputs(
        inputs["x"], inputs["w_attn"], inputs["b_attn"],
        inputs["w_proj"], inputs["b_proj"],
    )
    b_attn = np.asarray(inputs["b_attn"], dtype=np.float32)
    w_proj = np.asarray(inputs["w_proj"], dtype=np.float32)
    bvp = []
    for hg in range(2):
        cols = np.concatenate(
            [np.arange(h * D, (h + 1) * D)
             for h in range(hg * HPC, (hg + 1) * HPC)])
        bvp.append(b_attn[2 * C + cols] @ w_proj[cols, :])
    res = run_bass_kernel_spmd(nc, in_maps, core_ids=list(range(8)),
                               trace=trace)
    full = _postprocess([r["out"] for r in res.results],
                        inputs["b_proj"], bvp)
    return full, res


def kernel(**inputs):
    full, _ = _run(inputs, trace=False)
    return full


if __name__ == "__main__":
    _get_program()
    print("built ok")


# revision 12
# speedup vs baseline: 1.1637x; 1.1637x over previous
"""Trainium2 Bass kernel for causal self-attention (B=4, T=2048, C=2048, H=16).

Sharding: 8 cores = 4 batches x 2 head-groups (8 heads each).
All-fp16 datapath (PSUM accumulation fp32). Per core:
  A) v = x @ Wv            -> fp16 spill [T, 1024]
  B+C merged, per head h:
    B-block: qkT features (q_h, k_h) = Wqk^T x^T + RoPE -> resident fp16
    C-block: flash-style SDPA, t-outer/j-inner, software-pipelined:
      scores mm -> exp (scalar) -> [diag tri-mul] -> p_sum += p (vector)
      -> PV mm accumulates numerator in PSUM (tensor, lagged 2 items)
      per t: ones-mm denominator from p_sum -> recip_approx (vector)
      -> y = psy * rden resident fp16
  D) partial_out = y^T @ wp -> [T, C] fp16 partial
Host sums core pairs per batch, adds b_proj and the folded bias row
bv @ wp (token-independent).
"""

import sys

import numpy as np

sys.path.insert(0, "/opt/trn_rl_repo")

import concourse.bass as bass  # noqa: E402,F401
import concourse.mybir as mybir  # noqa: E402
import concourse.tile as tile  # noqa: E402
from concourse import bacc  # noqa: E402

F32 = mybir.dt.float32
F16 = mybir.dt.float16
AF = mybir.ActivationFunctionType

B, T, C = 4, 2048, 2048
H, D = 16, 128
HPC = 8            # heads per core
P = 128
NT = 512           # matmul moving free dim
TT = T // NT       # 4 token tiles
CC = C // P        # 16 contraction chunks over C
NF = 2 * HPC       # 16 feature chunks, interleaved (q_h, k_h) per head
ROPE_BASE = 10000.0

_CACHE = {}


def _mm(nc, out, lhsT, rhs, **kw):
    nc.tensor.matmul(out, lhsT, rhs, **kw)


def build_program():
    nc = bacc.Bacc(name="csa_tp3")

    xt = nc.dram_tensor("xt", (C, T), F16, kind="ExternalInput")
    wqk = nc.dram_tensor("wqk", (C, NF * P), F16, kind="ExternalInput")
    bqk = nc.dram_tensor("bqk", (P, NF), F32, kind="ExternalInput")
    wv = nc.dram_tensor("wv", (C, HPC * D), F16, kind="ExternalInput")
    cs = nc.dram_tensor("cs", (P, T), F16, kind="ExternalInput")
    sw = nc.dram_tensor("sw", (P, T), F16, kind="ExternalInput")
    tri = nc.dram_tensor("tri", (P, P), F16, kind="ExternalInput")
    onesm = nc.dram_tensor("onesm", (P, P), F16, kind="ExternalInput")
    wp = nc.dram_tensor("wp", (HPC * D, C), F16, kind="ExternalInput")
    out = nc.dram_tensor("out", (T, C), F16, kind="ExternalOutput")

    v_spill = nc.dram_tensor("v_spill", (T, HPC * D), F16, kind="Internal")
    y_spill = nc.dram_tensor("y_spill", (HPC * D, T), F16, kind="Internal")

    with tile.TileContext(nc) as tc:
        with tc.tile_pool(name="persist", bufs=1) as persist:
            # q/k resident: 16 features x [128, T] fp16 (64 KB/partition)
            qk_res = [
                persist.tile([P, T], F16, tag=f"qk{f}", name=f"qk{f}")
                for f in range(NF)
            ]
            cs_t = persist.tile([P, T], F16, tag="cs", name="cs")
            sw_t = persist.tile([P, T], F16, tag="sw", name="sw")
            bqk_t = persist.tile([P, NF], F32, tag="bqk", name="bqk")
            tri_t = persist.tile([P, P], F16, tag="tri", name="tri")
            ones_t = persist.tile([P, P], F16, tag="ones", name="ones")

            with tc.tile_pool(name="xt_res", bufs=1) as xt_res:
                xtt = [None] * CC  # [128, T] fp16 per c-chunk

                # ---------------- phase A: V (two n-passes) --------------
                with (
                    tc.tile_pool(name="wv_pool", bufs=1) as wv_pool,
                    tc.tile_pool(name="va_pool", bufs=1) as va_pool,
                    tc.tile_pool(name="psum_a", bufs=1, space="PSUM") as psum_a,
                ):
                    # interleaved issue: (wv_n0[c], xt[c]) pairs, split in
                    # half-columns so the first chain is fed ASAP
                    wvh = [[None] * CC, [None] * CC]
                    HN = NT // 2
                    for c in range(CC):
                        w_ = wv_pool.tile([P, NT], F16, tag=f"wva{c}",
                                          name=f"wva{c}")
                        x_ = xt_res.tile([P, T], F16, tag=f"x{c}",
                                         name=f"x{c}")
                        for hh in range(2):
                            nc.sync.dma_start(
                                w_[:, hh * HN:(hh + 1) * HN],
                                wv[c * P:(c + 1) * P, hh * HN:(hh + 1) * HN])
                            nc.sync.dma_start(
                                x_[:, hh * HN:(hh + 1) * HN],
                                xt[c * P:(c + 1) * P, hh * HN:(hh + 1) * HN])
                        wvh[0][c] = w_
                        xtt[c] = x_
                    # rest of x (t=1..3) in need-by order, then wv n=1
                    for tt_ in range(1, TT):
                        for c in range(CC):
                            nc.sync.dma_start(
                                xtt[c][:, tt_ * NT:(tt_ + 1) * NT],
                                xt[c * P:(c + 1) * P,
                                   tt_ * NT:(tt_ + 1) * NT])
                    for c in range(CC):
                        w_ = wv_pool.tile([P, NT], F16, tag=f"wvb{c}",
                                          name=f"wvb{c}")
                        nc.sync.dma_start(w_[:], wv[c * P:(c + 1) * P,
                                                    NT:2 * NT])
                        wvh[1][c] = w_
                    # constants
                    nc.sync.dma_start(cs_t[:], cs[:])
                    nc.sync.dma_start(sw_t[:], sw[:])
                    nc.sync.dma_start(bqk_t[:], bqk[:])
                    nc.sync.dma_start(tri_t[:], tri[:])
                    nc.sync.dma_start(ones_t[:], onesm[:])

                    for n in range(2):
                        for mtok in range(T // P):
                            msl = slice(mtok * P, (mtok + 1) * P)
                            ps = psum_a.tile([P, NT], F32, tag="psa",
                                             bufs=8, name="psa")
                            for c in range(CC):
                                _mm(nc, ps[:], xtt[c][:, msl], wvh[n][c][:],
                                    start=(c == 0), stop=(c == CC - 1))
                            vt = va_pool.tile([P, NT], F16, tag="vt",
                                              bufs=3, name="vt")
                            nc.scalar.copy(vt[:], ps[:])
                            nc.gpsimd.dma_start(
                                v_spill[mtok * P:(mtok + 1) * P,
                                        n * NT:(n + 1) * NT],
                                vt[:],
                            )

                # ------------- merged phases B + C, per head -------------
                with (
                    tc.tile_pool(name="wq_pool", bufs=1) as wq_pool,
                    tc.tile_pool(name="rp_pool", bufs=1) as rp_pool,
                    tc.tile_pool(name="vh_pool", bufs=1) as vh_pool,
                    tc.tile_pool(name="sd_pool", bufs=1) as sd_pool,
                    tc.tile_pool(name="psum_bc", bufs=1,
                                 space="PSUM") as psum_bc,
                ):
                    hd = D // 2

                    def load_wq(fg):
                        tiles = []
                        for c in range(CC):
                            w_ = wq_pool.tile([P, 2 * P], F16,
                                              tag=f"wq{c}", bufs=2,
                                              name=f"wq{c}")
                            nc.sync.dma_start(
                                w_[:],
                                wqk[c * P:(c + 1) * P,
                                    fg * 2 * P:(fg + 1) * 2 * P],
                            )
                            tiles.append(w_)
                        return tiles

                    vh_t = [None] * HPC

                    def load_vh(h):
                        vh3 = vh_pool.tile([P, T // P, P], F16,
                                           tag="vh", bufs=3, name="vh3")
                        nc.sync.dma_start(
                            vh3[:],
                            v_spill[:, h * D:(h + 1) * D].rearrange(
                                "(j p) d -> p j d", p=P),
                        )
                        vh_t[h] = vh3

                    # C-block software pipeline (global across heads)
                    state = {}    # (h,t) -> (psy, p_sum)
                    pending = []  # [(h,t,j,nj,p,off)]
                    LOOK = 2

                    def c_front(h, t, j, nj):
                        qh = qk_res[2 * h]
                        kh = qk_res[2 * h + 1]
                        diag = (j >= 4 * t)
                        off = (j - 4 * t) * P if diag else 0
                        qsl = slice(t * NT + off, (t + 1) * NT)
                        pss = psum_bc.tile([P, NT], F32, tag="pss",
                                           bufs=3, name="pss")
                        _mm(nc, pss[:, off:],
                            kh[:, j * P:(j + 1) * P],
                            qh[:, qsl], start=True, stop=True)
                        p = sd_pool.tile([P, NT], F16, tag="p",
                                         bufs=5, name="p")
                        nc.scalar.activation(
                            p[:, off:], pss[:, off:], AF.Exp)
                        if diag:
                            nc.vector.tensor_mul(
                                p[:, off:off + P],
                                p[:, off:off + P],
                                tri_t[:],
                            )
                        if j == 0:
                            psy = psum_bc.tile([P, NT], F32, tag="psy",
                                               bufs=2, name="psy")
                            p_sum = sd_pool.tile([P, NT], F16,
                                                 tag="p_sum", bufs=2,
                                                 name="p_sum")
                            state[(h, t)] = (psy, p_sum)
                            nc.vector.tensor_copy(state[(h, t)][1][:], p[:])
                        else:
                            p_sum = state[(h, t)][1]
                            nc.vector.tensor_add(
                                p_sum[:, off:], p_sum[:, off:], p[:, off:])
                        pending.append((h, t, j, nj, p, off))

                    def c_back():
                        h, t, j, nj, p, off = pending.pop(0)
                        psy, p_sum = state[(h, t)]
                        _mm(nc, psy[:, off:],
                            vh_t[h][:, j, :], p[:, off:],
                            start=(j == 0), stop=(j == nj - 1))
                        if j == nj - 1:
                            psd = psum_bc.tile([P, NT], F32, tag="psd",
                                               bufs=1, name="psd")
                            _mm(nc, psd[:], ones_t[:], p_sum[:],
                                start=True, stop=True)
                            rden = sd_pool.tile([P, NT], F32,
                                                tag="rden", bufs=2,
                                                name="rden")
                            nc.vector.reciprocal_approx_fast(
                                rden[:], psd[:])
                            yst = sd_pool.tile([P, NT], F16,
                                                tag="yst", bufs=2,
                                                name="yst")
                            nc.vector.tensor_mul(yst[:], psy[:], rden[:])
                            nc.gpsimd.dma_start(
                                y_spill[h * P:(h + 1) * P,
                                        t * NT:(t + 1) * NT],
                                yst[:])
                            del state[(h, t)]

                    def chain(wq_t, h, f, t):
                        """One B-chain (16 mms) + RoPE for feature tile t."""
                        feat = h * 2 + f
                        ps = psum_bc.tile([P, NT], F32, tag="psb",
                                          bufs=2, name="psb")
                        for c in range(CC):
                            _mm(nc, ps[:],
                                wq_t[c][:, f * P:(f + 1) * P],
                                xtt[c][:, t * NT:(t + 1) * NT],
                                start=(c == 0), stop=(c == CC - 1))
                        sl = slice(t * NT, (t + 1) * NT)
                        raw = rp_pool.tile([P, NT], F16, tag="raw",
                                           bufs=2, name="raw")
                        nc.scalar.activation(
                            raw[:], ps[:], AF.Identity,
                            bias=bqk_t[:, feat:feat + 1],
                        )
                        rsw = rp_pool.tile([P, NT], F16, tag="rsw",
                                           bufs=2, name="rsw")
                        nc.scalar.activation(
                            rsw[0:hd, :], ps[hd:P, :], AF.Identity,
                            bias=bqk_t[hd:P, feat:feat + 1],
                        )
                        nc.scalar.activation(
                            rsw[hd:P, :], ps[0:hd, :], AF.Identity,
                            bias=bqk_t[0:hd, feat:feat + 1],
                        )
                        t1 = rp_pool.tile([P, NT], F16, tag="rt1",
                                          bufs=2, name="rt1")
                        t2 = rp_pool.tile([P, NT], F16, tag="rt2",
                                          bufs=2, name="rt2")
                        nc.vector.tensor_mul(t1[:], raw[:], cs_t[:, sl])
                        nc.vector.tensor_mul(t2[:], rsw[:], sw_t[:, sl])
                        nc.vector.tensor_add(
                            qk_res[feat][:, sl], t1[:], t2[:])

                    def c_group(h, t):
                        nj = 4 * t + 4
                        for j in range(nj):
                            c_front(h, t, j, nj)
                            if len(pending) > LOOK:
                                c_back()

                    wq_next = load_wq(0)
                    load_vh(0)
                    load_vh(1)
                    for h in range(HPC):
                        # interleave qk-projection chains with SDPA groups
                        # so the tensor engine never waits on RoPE drains
                        wq_t = wq_next
                        if h + 1 < HPC:
                            wq_next = load_wq(h + 1)
                        if h + 2 < HPC:
                            load_vh(h + 2)
                        chain(wq_t, h, 0, 0)
                        chain(wq_t, h, 0, 1)
                        chain(wq_t, h, 1, 0)
                        chain(wq_t, h, 0, 2)
                        c_group(h, 0)
                        chain(wq_t, h, 0, 3)
                        chain(wq_t, h, 1, 1)
                        c_group(h, 1)
                        chain(wq_t, h, 1, 2)
                        c_group(h, 2)
                        chain(wq_t, h, 1, 3)
                        c_group(h, 3)
                    while pending:
                        c_back()

            # ------------- phase D: projection -------------
            with (
                tc.tile_pool(name="wp_pool", bufs=1) as wp_pool,
                tc.tile_pool(name="ym_pool", bufs=1) as ym_pool,
                tc.tile_pool(name="ot_pool", bufs=1) as ot_pool,
                tc.tile_pool(name="psum_d", bufs=1, space="PSUM") as psum_d,
            ):
                wp_t = []
                for hh in range(HPC):
                    w_ = wp_pool.tile([P, C], F16, tag=f"wp{hh}",
                                      name=f"wp{hh}")
                    for n in range(4):
                        nc.sync.dma_start(
                            w_[:, n * NT:(n + 1) * NT],
                            wp[hh * P:(hh + 1) * P, n * NT:(n + 1) * NT])
                    wp_t.append(w_)

                ym_t = [None] * (T // P)

                def load_ym(m):
                    ym = ym_pool.tile([P, HPC, P], F16, tag="ym",
                                      bufs=3, name="ym")
                    nc.sync.dma_start(
                        ym[:],
                        y_spill[:, m * P:(m + 1) * P].rearrange(
                            "(h d) t -> d h t", d=P),
                    )
                    ym_t[m] = ym

                load_ym(0)
                load_ym(1)
                for m in range(T // P):
                    if m + 2 < T // P:
                        load_ym(m + 2)
                    msl = slice(m * P, (m + 1) * P)
                    pso = [
                        psum_d.tile([P, NT], F32, tag=f"pso{n}",
                                    bufs=2, name=f"pso{n}")
                        for n in range(4)
                    ]
                    for hh in range(HPC):
                        lhsT = ym_t[m][:, hh, :]
                        for n in range(4):
                            _mm(nc, pso[n][:], lhsT,
                                wp_t[hh][:, n * NT:(n + 1) * NT],
                                start=(hh == 0),
                                stop=(hh == HPC - 1))
                    ot = ot_pool.tile([P, C], F16, tag="ot",
                                      bufs=2, name="ot")
                    for n in range(4):
                        nc.scalar.copy(
                            ot[:, n * NT:(n + 1) * NT], pso[n][:])
                        nc.gpsimd.dma_start(
                            out[msl, n * NT:(n + 1) * NT],
                            ot[:, n * NT:(n + 1) * NT])

    nc.finalize()
    return nc


def prep_inputs(x, w_attn, b_attn, w_proj, b_proj):
    """Build the 8 per-core input maps from full inputs."""
    x = np.asarray(x, dtype=np.float32)
    w_attn = np.asarray(w_attn, dtype=np.float32)
    b_attn = np.asarray(b_attn, dtype=np.float32)
    w_proj = np.asarray(w_proj, dtype=np.float32)

    scale = np.float32(1.0 / np.sqrt(D))

    inv_freq = 1.0 / (ROPE_BASE ** (np.arange(0, D, 2, dtype=np.float32) / D))
    tpos = np.arange(T, dtype=np.float32)
    ang = np.outer(tpos, inv_freq)  # [T, 64]
    cos_t, sin_t = np.cos(ang).T, np.sin(ang).T  # [64, T]
    cs = np.ascontiguousarray(
        np.concatenate([cos_t, cos_t], axis=0)).astype(np.float16)
    sw = np.ascontiguousarray(
        np.concatenate([-sin_t, sin_t], axis=0)).astype(np.float16)

    qq = np.arange(P)
    kk = np.arange(P)[:, None]
    tri = np.ascontiguousarray(
        (qq[None, :] >= kk).astype(np.float16))  # [128,128] causal triangle

    onesm = np.ones((P, P), dtype=np.float16)

    in_maps = []
    for core in range(8):
        b = core // 2
        hg = core % 2
        heads = list(range(hg * HPC, (hg + 1) * HPC))
        # interleaved feature order: (q_h, k_h) per head
        wqk_cols = []
        bqk_vals = []
        for h in heads:
            qcol = np.arange(h * D, (h + 1) * D)
            kcol = qcol + C
            wqk_cols.append(w_attn[:, qcol] * scale)
            wqk_cols.append(w_attn[:, kcol])
            bqk_vals.append(b_attn[qcol] * scale)
            bqk_vals.append(b_attn[kcol])
        wqk_s = np.ascontiguousarray(
            np.concatenate(wqk_cols, axis=1)).astype(np.float16)
        bqk_s = np.ascontiguousarray(
            np.stack(bqk_vals, axis=1)).astype(np.float32)  # [128, 16]

        vcols = np.concatenate(
            [np.arange(h * D, (h + 1) * D) for h in heads]) + 2 * C
        wv_s = np.ascontiguousarray(w_attn[:, vcols]).astype(np.float16)
        pcols = np.concatenate(
            [np.arange(h * D, (h + 1) * D) for h in heads])
        wp_s = np.ascontiguousarray(w_proj[pcols, :]).astype(np.float16)
        xt_s = np.ascontiguousarray(x[b].T).astype(np.float16)

        in_maps.append({
            "xt": xt_s, "wqk": wqk_s, "bqk": bqk_s, "wv": wv_s,
            "cs": cs, "sw": sw, "tri": tri, "onesm": onesm, "wp": wp_s,
        })
    return in_maps


def _get_program():
    if "nc" not in _CACHE:
        _CACHE["nc"] = build_program()
    return _CACHE["nc"]


def _postprocess(outs, b_proj, bvp):
    # bvp[hg]: bv_core @ wp_core for head-group hg — the attention value
    # bias contributes a token-independent row to the projection output.
    base = np.asarray(b_proj, dtype=np.float32) + bvp[0] + bvp[1]
    return np.stack(
        [outs[2 * b].astype(np.float32) + outs[2 * b + 1].astype(np.float32)
         + base[None, :] for b in range(B)]
    ).astype(np.float32)


def _run(inputs, trace=False):
    from concourse.bass_utils import run_bass_kernel_spmd

    nc = _get_program()
    in_maps = prep_inputs(
        inputs["x"], inputs["w_attn"], inputs["b_attn"],
        inputs["w_proj"], inputs["b_proj"],
    )
    b_attn = np.asarray(inputs["b_attn"], dtype=np.float32)
    w_proj = np.asarray(inputs["w_proj"], dtype=np.float32)
    bvp = []
    for hg in range(2):
        cols = np.concatenate(
            [np.arange(h * D, (h + 1) * D)
             for h in range(hg * HPC, (hg + 1) * HPC)])
        bvp.append(b_attn[2 * C + cols] @ w_proj[cols, :])
    res = run_bass_kernel_spmd(nc, in_maps, core_ids=list(range(8)),
                               trace=trace)
    full = _postprocess([r["out"] for r in res.results],
                        inputs["b_proj"], bvp)
    return full, res


def kernel(**inputs):
    full, _ = _run(inputs, trace=False)
    return full


if __name__ == "__main__":
    _get_program()
    print("built ok")


# revision 13
# speedup vs baseline: 1.1794x; 1.0135x over previous
"""Trainium2 Bass kernel for causal self-attention (B=4, T=2048, C=2048, H=16).

Sharding: 8 cores = 4 batches x 2 head-groups (8 heads each).
All-fp16 datapath (PSUM accumulation fp32). Per core:
  A) v = x @ Wv            -> fp16 spill [T, 1024]
  B+C merged, per head h:
    B-block: qkT features (q_h, k_h) = Wqk^T x^T + RoPE -> resident fp16
    C-block: flash-style SDPA, t-outer/j-inner, software-pipelined:
      scores mm -> exp (scalar) -> [diag tri-mul] -> p_sum += p (vector)
      -> PV mm accumulates numerator in PSUM (tensor, lagged 2 items)
      per t: ones-mm denominator from p_sum -> recip_approx (vector)
      -> y = psy * rden resident fp16
  D) partial_out = y^T @ wp -> [T, C] fp16 partial
Host sums core pairs per batch, adds b_proj and the folded bias row
bv @ wp (token-independent).
"""

import sys

import numpy as np

sys.path.insert(0, "/opt/trn_rl_repo")

import concourse.bass as bass  # noqa: E402,F401
import concourse.mybir as mybir  # noqa: E402
import concourse.tile as tile  # noqa: E402
from concourse import bacc  # noqa: E402

F32 = mybir.dt.float32
F16 = mybir.dt.float16
AF = mybir.ActivationFunctionType

B, T, C = 4, 2048, 2048
H, D = 16, 128
HPC = 8            # heads per core
P = 128
NT = 512           # matmul moving free dim
TT = T // NT       # 4 token tiles
CC = C // P        # 16 contraction chunks over C
NF = 2 * HPC       # 16 feature chunks, interleaved (q_h, k_h) per head
ROPE_BASE = 10000.0

_CACHE = {}


def _mm(nc, out, lhsT, rhs, **kw):
    nc.tensor.matmul(out, lhsT, rhs, **kw)


def build_program():
    nc = bacc.Bacc(name="csa_tp3")

    xt = nc.dram_tensor("xt", (C, T), F16, kind="ExternalInput")
    wqk = nc.dram_tensor("wqk", (C, NF * P), F16, kind="ExternalInput")
    bqk = nc.dram_tensor("bqk", (P, NF), F32, kind="ExternalInput")
    wv = nc.dram_tensor("wv", (C, HPC * D), F16, kind="ExternalInput")
    cs = nc.dram_tensor("cs", (P, T), F16, kind="ExternalInput")
    sw = nc.dram_tensor("sw", (P, T), F16, kind="ExternalInput")
    tri = nc.dram_tensor("tri", (P, P), F16, kind="ExternalInput")
    onesm = nc.dram_tensor("onesm", (P, P), F16, kind="ExternalInput")
    wp = nc.dram_tensor("wp", (HPC * D, C), F16, kind="ExternalInput")
    out = nc.dram_tensor("out", (T, C), F16, kind="ExternalOutput")

    v_spill = nc.dram_tensor("v_spill", (T, HPC * D), F16, kind="Internal")
    y_spill = nc.dram_tensor("y_spill", (HPC * D, T), F16, kind="Internal")

    with tile.TileContext(nc) as tc:
        with tc.tile_pool(name="persist", bufs=1) as persist:
            # q/k resident: 16 features x [128, T] fp16 (64 KB/partition)
            qk_res = [
                persist.tile([P, T], F16, tag=f"qk{f}", name=f"qk{f}")
                for f in range(NF)
            ]
            cs_t = persist.tile([P, T], F16, tag="cs", name="cs")
            sw_t = persist.tile([P, T], F16, tag="sw", name="sw")
            bqk_t = persist.tile([P, NF], F32, tag="bqk", name="bqk")
            tri_t = persist.tile([P, P], F16, tag="tri", name="tri")
            ones_t = persist.tile([P, P], F16, tag="ones", name="ones")

            with (
                tc.tile_pool(name="xt_res", bufs=1) as xt_res,
                tc.tile_pool(name="wq_pool", bufs=1) as wq_pool,
            ):
                xtt = [None] * CC  # [128, T] fp16 per c-chunk

                def load_wq(fg):
                    tiles = []
                    for c in range(CC):
                        w_ = wq_pool.tile([P, 2 * P], F16,
                                          tag=f"wq{c}", bufs=2,
                                          name=f"wq{c}")
                        nc.sync.dma_start(
                            w_[:],
                            wqk[c * P:(c + 1) * P,
                                fg * 2 * P:(fg + 1) * 2 * P],
                        )
                        tiles.append(w_)
                    return tiles

                # ---------------- phase A: V (two n-passes) --------------
                with (
                    tc.tile_pool(name="wv_pool", bufs=1) as wv_pool,
                    tc.tile_pool(name="va_pool", bufs=1) as va_pool,
                    tc.tile_pool(name="psum_a", bufs=1, space="PSUM") as psum_a,
                ):
                    # interleaved issue: (wva[c], xt_t0[c], wvb[c])
                    # triplets split in half-columns; t-outer consumption
                    # gives each later xt t-slice 8 chains of slack
                    wvh = [[None] * CC, [None] * CC]
                    HN = NT // 2
                    for c in range(CC):
                        wa = wv_pool.tile([P, NT], F16, tag=f"wva{c}",
                                          name=f"wva{c}")
                        wb = wv_pool.tile([P, NT], F16, tag=f"wvb{c}",
                                          name=f"wvb{c}")
                        x_ = xt_res.tile([P, T], F16, tag=f"x{c}",
                                         name=f"x{c}")
                        for hh in range(2):
                            hsl = slice(hh * HN, (hh + 1) * HN)
                            nc.sync.dma_start(
                                wa[:, hsl], wv[c * P:(c + 1) * P, hsl])
                            nc.sync.dma_start(
                                x_[:, hsl], xt[c * P:(c + 1) * P, hsl])
                            nc.sync.dma_start(
                                wb[:, hsl],
                                wv[c * P:(c + 1) * P,
                                   NT + hh * HN:NT + (hh + 1) * HN])
                        wvh[0][c] = wa
                        wvh[1][c] = wb
                        xtt[c] = x_
                    for tt_ in range(1, TT):
                        for c in range(CC):
                            nc.sync.dma_start(
                                xtt[c][:, tt_ * NT:(tt_ + 1) * NT],
                                xt[c * P:(c + 1) * P,
                                   tt_ * NT:(tt_ + 1) * NT])
                    # constants
                    nc.sync.dma_start(cs_t[:], cs[:])
                    nc.sync.dma_start(sw_t[:], sw[:])
                    nc.sync.dma_start(bqk_t[:], bqk[:])
                    nc.sync.dma_start(tri_t[:], tri[:])
                    nc.sync.dma_start(ones_t[:], onesm[:])
                    # prefetch first head's qk weights during phase A
                    wq_first = load_wq(0)

                    for tt_ in range(TT):
                        for n in range(2):
                            for m in range(4):
                                mtok = tt_ * 4 + m
                                msl = slice(mtok * P, (mtok + 1) * P)
                                ps = psum_a.tile([P, NT], F32, tag="psa",
                                                 bufs=8, name="psa")
                                for c in range(CC):
                                    _mm(nc, ps[:], xtt[c][:, msl],
                                        wvh[n][c][:],
                                        start=(c == 0), stop=(c == CC - 1))
                                vt = va_pool.tile([P, NT], F16, tag="vt",
                                                  bufs=3, name="vt")
                                nc.scalar.copy(vt[:], ps[:])
                                nc.gpsimd.dma_start(
                                    v_spill[mtok * P:(mtok + 1) * P,
                                            n * NT:(n + 1) * NT],
                                    vt[:],
                                )

                # ------------- merged phases B + C, per head -------------
                with (
                    tc.tile_pool(name="rp_pool", bufs=1) as rp_pool,
                    tc.tile_pool(name="vh_pool", bufs=1) as vh_pool,
                    tc.tile_pool(name="sd_pool", bufs=1) as sd_pool,
                    tc.tile_pool(name="psum_bc", bufs=1,
                                 space="PSUM") as psum_bc,
                ):
                    hd = D // 2

                    vh_t = [None] * HPC

                    def load_vh(h):
                        vh3 = vh_pool.tile([P, T // P, P], F16,
                                           tag="vh", bufs=3, name="vh3")
                        nc.sync.dma_start(
                            vh3[:],
                            v_spill[:, h * D:(h + 1) * D].rearrange(
                                "(j p) d -> p j d", p=P),
                        )
                        vh_t[h] = vh3

                    # C-block software pipeline (global across heads)
                    state = {}    # (h,t) -> (psy, p_sum)
                    pending = []  # [(h,t,j,nj,p,off)]
                    LOOK = 2

                    def c_front(h, t, j, nj):
                        qh = qk_res[2 * h]
                        kh = qk_res[2 * h + 1]
                        diag = (j >= 4 * t)
                        off = (j - 4 * t) * P if diag else 0
                        qsl = slice(t * NT + off, (t + 1) * NT)
                        pss = psum_bc.tile([P, NT], F32, tag="pss",
                                           bufs=3, name="pss")
                        _mm(nc, pss[:, off:],
                            kh[:, j * P:(j + 1) * P],
                            qh[:, qsl], start=True, stop=True)
                        p = sd_pool.tile([P, NT], F16, tag="p",
                                         bufs=5, name="p")
                        nc.scalar.activation(
                            p[:, off:], pss[:, off:], AF.Exp)
                        if diag:
                            nc.vector.tensor_mul(
                                p[:, off:off + P],
                                p[:, off:off + P],
                                tri_t[:],
                            )
                        if j == 0:
                            psy = psum_bc.tile([P, NT], F32, tag="psy",
                                               bufs=2, name="psy")
                            p_sum = sd_pool.tile([P, NT], F16,
                                                 tag="p_sum", bufs=2,
                                                 name="p_sum")
                            state[(h, t)] = (psy, p_sum)
                            nc.vector.tensor_copy(state[(h, t)][1][:], p[:])
                        else:
                            p_sum = state[(h, t)][1]
                            nc.vector.tensor_add(
                                p_sum[:, off:], p_sum[:, off:], p[:, off:])
                        pending.append((h, t, j, nj, p, off))

                    def c_back():
                        h, t, j, nj, p, off = pending.pop(0)
                        psy, p_sum = state[(h, t)]
                        _mm(nc, psy[:, off:],
                            vh_t[h][:, j, :], p[:, off:],
                            start=(j == 0), stop=(j == nj - 1))
                        if j == nj - 1:
                            psd = psum_bc.tile([P, NT], F32, tag="psd",
                                               bufs=1, name="psd")
                            _mm(nc, psd[:], ones_t[:], p_sum[:],
                                start=True, stop=True)
                            rden = sd_pool.tile([P, NT], F32,
                                                tag="rden", bufs=2,
                                                name="rden")
                            nc.vector.reciprocal_approx_fast(
                                rden[:], psd[:])
                            yst = sd_pool.tile([P, NT], F16,
                                                tag="yst", bufs=2,
                                                name="yst")
                            nc.vector.tensor_mul(yst[:], psy[:], rden[:])
                            nc.gpsimd.dma_start(
                                y_spill[h * P:(h + 1) * P,
                                        t * NT:(t + 1) * NT],
                                yst[:])
                            del state[(h, t)]

                    def chain(wq_t, h, f, t):
                        """One B-chain (16 mms) + RoPE for feature tile t."""
                        feat = h * 2 + f
                        ps = psum_bc.tile([P, NT], F32, tag="psb",
                                          bufs=2, name="psb")
                        for c in range(CC):
                            _mm(nc, ps[:],
                                wq_t[c][:, f * P:(f + 1) * P],
                                xtt[c][:, t * NT:(t + 1) * NT],
                                start=(c == 0), stop=(c == CC - 1))
                        sl = slice(t * NT, (t + 1) * NT)
                        raw = rp_pool.tile([P, NT], F16, tag="raw",
                                           bufs=2, name="raw")
                        nc.scalar.activation(
                            raw[:], ps[:], AF.Identity,
                            bias=bqk_t[:, feat:feat + 1],
                        )
                        rsw = rp_pool.tile([P, NT], F16, tag="rsw",
                                           bufs=2, name="rsw")
                        nc.scalar.activation(
                            rsw[0:hd, :], ps[hd:P, :], AF.Identity,
                            bias=bqk_t[hd:P, feat:feat + 1],
                        )
                        nc.scalar.activation(
                            rsw[hd:P, :], ps[0:hd, :], AF.Identity,
                            bias=bqk_t[0:hd, feat:feat + 1],
                        )
                        t1 = rp_pool.tile([P, NT], F16, tag="rt1",
                                          bufs=2, name="rt1")
                        t2 = rp_pool.tile([P, NT], F16, tag="rt2",
                                          bufs=2, name="rt2")
                        nc.vector.tensor_mul(t1[:], raw[:], cs_t[:, sl])
                        nc.vector.tensor_mul(t2[:], rsw[:], sw_t[:, sl])
                        nc.vector.tensor_add(
                            qk_res[feat][:, sl], t1[:], t2[:])

                    def c_group(h, t):
                        nj = 4 * t + 4
                        for j in range(nj):
                            c_front(h, t, j, nj)
                            if len(pending) > LOOK:
                                c_back()

                    wq_next = wq_first
                    load_vh(0)
                    load_vh(1)
                    for h in range(HPC):
                        # interleave qk-projection chains with SDPA groups
                        # so the tensor engine never waits on RoPE drains
                        wq_t = wq_next
                        if h + 1 < HPC:
                            wq_next = load_wq(h + 1)
                        if h + 2 < HPC:
                            load_vh(h + 2)
                        chain(wq_t, h, 1, 0)
                        chain(wq_t, h, 0, 0)
                        chain(wq_t, h, 0, 1)
                        chain(wq_t, h, 0, 2)
                        c_group(h, 0)
                        chain(wq_t, h, 0, 3)
                        chain(wq_t, h, 1, 1)
                        c_group(h, 1)
                        chain(wq_t, h, 1, 2)
                        c_group(h, 2)
                        chain(wq_t, h, 1, 3)
                        c_group(h, 3)
                    while pending:
                        c_back()

            # ------------- phase D: projection -------------
            with (
                tc.tile_pool(name="wp_pool", bufs=1) as wp_pool,
                tc.tile_pool(name="ym_pool", bufs=1) as ym_pool,
                tc.tile_pool(name="ot_pool", bufs=1) as ot_pool,
                tc.tile_pool(name="psum_d", bufs=1, space="PSUM") as psum_d,
            ):
                ym_t = [None] * (T // P)

                def load_ym(m):
                    ym = ym_pool.tile([P, HPC, P], F16, tag="ym",
                                      bufs=3, name="ym")
                    nc.sync.dma_start(
                        ym[:],
                        y_spill[:, m * P:(m + 1) * P].rearrange(
                            "(h d) t -> d h t", d=P),
                    )
                    ym_t[m] = ym

                load_ym(0)
                load_ym(1)
                wp_t = []
                for hh in range(HPC):
                    w_ = wp_pool.tile([P, C], F16, tag=f"wp{hh}",
                                      name=f"wp{hh}")
                    for n in range(4):
                        nc.sync.dma_start(
                            w_[:, n * NT:(n + 1) * NT],
                            wp[hh * P:(hh + 1) * P, n * NT:(n + 1) * NT])
                    wp_t.append(w_)
                for m in range(T // P):
                    if m + 2 < T // P:
                        load_ym(m + 2)
                    msl = slice(m * P, (m + 1) * P)
                    pso = [
                        psum_d.tile([P, NT], F32, tag=f"pso{n}",
                                    bufs=2, name=f"pso{n}")
                        for n in range(4)
                    ]
                    for hh in range(HPC):
                        lhsT = ym_t[m][:, hh, :]
                        for n in range(4):
                            _mm(nc, pso[n][:], lhsT,
                                wp_t[hh][:, n * NT:(n + 1) * NT],
                                start=(hh == 0),
                                stop=(hh == HPC - 1))
                    ot = ot_pool.tile([P, C], F16, tag="ot",
                                      bufs=2, name="ot")
                    for n in range(4):
                        nc.scalar.copy(
                            ot[:, n * NT:(n + 1) * NT], pso[n][:])
                        nc.gpsimd.dma_start(
                            out[msl, n * NT:(n + 1) * NT],
                            ot[:, n * NT:(n + 1) * NT])

    nc.finalize()
    return nc


def prep_inputs(x, w_attn, b_attn, w_proj, b_proj):
    """Build the 8 per-core input maps from full inputs."""
    x = np.asarray(x, dtype=np.float32)
    w_attn = np.asarray(w_attn, dtype=np.float32)
    b_attn = np.asarray(b_attn, dtype=np.float32)
    w_proj = np.asarray(w_proj, dtype=np.float32)

    scale = np.float32(1.0 / np.sqrt(D))

    inv_freq = 1.0 / (ROPE_BASE ** (np.arange(0, D, 2, dtype=np.float32) / D))
    tpos = np.arange(T, dtype=np.float32)
    ang = np.outer(tpos, inv_freq)  # [T, 64]
    cos_t, sin_t = np.cos(ang).T, np.sin(ang).T  # [64, T]
    cs = np.ascontiguousarray(
        np.concatenate([cos_t, cos_t], axis=0)).astype(np.float16)
    sw = np.ascontiguousarray(
        np.concatenate([-sin_t, sin_t], axis=0)).astype(np.float16)

    qq = np.arange(P)
    kk = np.arange(P)[:, None]
    tri = np.ascontiguousarray(
        (qq[None, :] >= kk).astype(np.float16))  # [128,128] causal triangle

    onesm = np.ones((P, P), dtype=np.float16)

    in_maps = []
    for core in range(8):
        b = core // 2
        hg = core % 2
        heads = list(range(hg * HPC, (hg + 1) * HPC))
        # interleaved feature order: (q_h, k_h) per head
        wqk_cols = []
        bqk_vals = []
        for h in heads:
            qcol = np.arange(h * D, (h + 1) * D)
            kcol = qcol + C
            wqk_cols.append(w_attn[:, qcol] * scale)
            wqk_cols.append(w_attn[:, kcol])
            bqk_vals.append(b_attn[qcol] * scale)
            bqk_vals.append(b_attn[kcol])
        wqk_s = np.ascontiguousarray(
            np.concatenate(wqk_cols, axis=1)).astype(np.float16)
        bqk_s = np.ascontiguousarray(
            np.stack(bqk_vals, axis=1)).astype(np.float32)  # [128, 16]

        vcols = np.concatenate(
            [np.arange(h * D, (h + 1) * D) for h in heads]) + 2 * C
        wv_s = np.ascontiguousarray(w_attn[:, vcols]).astype(np.float16)
        pcols = np.concatenate(
            [np.arange(h * D, (h + 1) * D) for h in heads])
        wp_s = np.ascontiguousarray(w_proj[pcols, :]).astype(np.float16)
        xt_s = np.ascontiguousarray(x[b].T).astype(np.float16)

        in_maps.append({
            "xt": xt_s, "wqk": wqk_s, "bqk": bqk_s, "wv": wv_s,
            "cs": cs, "sw": sw, "tri": tri, "onesm": onesm, "wp": wp_s,
        })
    return in_maps


def _get_program():
    if "nc" not in _CACHE:
        _CACHE["nc"] = build_program()
    return _CACHE["nc"]


def _postprocess(outs, b_proj, bvp):
    # bvp[hg]: bv_core @ wp_core for head-group hg — the attention value
    # bias contributes a token-independent row to the projection output.
    base = np.asarray(b_proj, dtype=np.float32) + bvp[0] + bvp[1]
    return np.stack(
        [outs[2 * b].astype(np.float32) + outs[2 * b + 1].astype(np.float32)
         + base[None, :] for b in range(B)]
    ).astype(np.float32)


def _run(inputs, trace=False):
    from concourse.bass_utils import run_bass_kernel_spmd

    nc = _get_program()
    in_maps = prep_inputs(
        inputs["x"], inputs["w_attn"], inputs["b_attn"],
        inputs["w_proj"], inputs["b_proj"],
    )
    b_attn = np.asarray(inputs["b_attn"], dtype=np.float32)
    w_proj = np.asarray(inputs["w_proj"], dtype=np.float32)
    bvp = []
    for hg in range(2):
        cols = np.concatenate(
            [np.arange(h * D, (h + 1) * D)
             for h in range(hg * HPC, (hg + 1) * HPC)])
        bvp.append(b_attn[2 * C + cols] @ w_proj[cols, :])
    res = run_bass_kernel_spmd(nc, in_maps, core_ids=list(range(8)),
                               trace=trace)
    full = _postprocess([r["out"] for r in res.results],
                        inputs["b_proj"], bvp)
    return full, res


def kernel(**inputs):
    full, _ = _run(inputs, trace=False)
    return full


if __name__ == "__main__":
    _get_program()
    print("built ok")
